# revision 3
# baseline (speedup 1.0000x reference)
"""GCN layer (Linear -> weighted-adjacency SpMM -> BatchNorm(eval) -> exact GELU)
as a Bass/Tile kernel on 8 Trainium2 NeuronCores.

Sharding: destination-node rows are sharded across the 8 cores (12500 rows
each); edges partitioned by destination row; W/b/BN params replicated.  Each
core computes the full `support = x @ W' + b'` redundantly in bf16 (phase 1:
cheaper than collectives at this size), writing it to HBM as 4
int16-addressable row-sections.  Phase 2 walks destination tiles (128 rows
each): per tile, four dma_gathers (one per section, per-gather edge count in a
runtime register trimming descriptor generation to real edges) fetch source
rows, and 4*c_sub one-hot selector matmuls plus one folded BN-shift matmul and
one bias matmul accumulate the entire segment-sum into a single PSUM group.

Engine allocation is strictly partitioned so no in-order queue ever blocks
another: PE does all matmuls; DVE only builds selectors; the scalar engine
drains PSUM (phase-1 copy, phase-2 exact GELU) and issues support/output
writes on its own HWDGE ring (parallel to the sync ring doing loads); gpsimd
preps gathers round-robin across all 4 SWDGE queues (queue = source section,
so a queue only starts once phase 1 has produced its section - measured 4x
descriptor throughput vs a single queue, which is the kernel's floor:
~400k edges/core x ~3.3ns/descriptor).

Host-side prep inside kernel(): repack x into a per-supertile
contiguous-per-partition layout; permute support rows so phase-1 writes are
contiguous per partition (the gather indices absorb the permutation); group
edges by (dest-tile, source-section) and pack per-group edge row/val into
fixed 128-edge chunk layout plus 16-partition-wrapped int16 gather indices.
BN is folded on the host (W' = W * s, shift = beta - mean * s,
s = gamma / sqrt(var + eps)).  One SPMD program serves all 8 cores.
"""

import sys

sys.path.insert(0, "/opt/trn_rl_repo")

import numpy as np
import ml_dtypes

import concourse.tile as tile
from concourse import bacc, mybir
from concourse.bass_utils import run_bass_kernel_spmd

F32 = mybir.dt.float32
BF16 = mybir.dt.bfloat16
I16 = mybir.dt.int16
AF = mybir.ActivationFunctionType
ALU = mybir.AluOpType

BF = ml_dtypes.bfloat16

N_CORES = 8
TPS = 14      # dest tiles per index-slab load (must divide nt)
XCOLS = 3584  # node columns per phase-1 supertile (divides sec_rows)
NGBUF = 3     # round-robin gather buffers (one per dest tile in flight)


def _build_program(*, in_dim, out_dim, npad, nt, c_sub, tps, xcols):
    assert in_dim % 128 == 0 and npad % (4 * xcols) == 0
    assert nt % tps == 0
    kb = in_dim // 128
    sec_rows = npad // 4
    nsup_sec = sec_rows // xcols
    jt = xcols // 128
    nidx = c_sub * 128
    idxcols = nidx // 16
    cs4 = 4 * c_sub

    nc = bacc.Bacc("TRN2", target_bir_lowering=False, debug=False,
                   num_devices=N_CORES, num_swdge_queues=4)

    nsup = npad // xcols
    xTa = nc.dram_tensor("xTa", [128, nsup * in_dim // 128 * xcols], BF16,
                         kind="ExternalInput").ap()
    Wp = nc.dram_tensor("Wp", [in_dim, out_dim], BF16, kind="ExternalInput").ap()
    bp = nc.dram_tensor("bp", [128, out_dim], BF16, kind="ExternalInput").ap()
    shiftb = nc.dram_tensor("shiftb", [128, out_dim], BF16, kind="ExternalInput").ap()
    ones0_in = nc.dram_tensor("ones0", [128, 128], BF16, kind="ExternalInput").ap()
    iota_in = nc.dram_tensor("iota", [128, 128], BF16, kind="ExternalInput").ap()
    idxp = nc.dram_tensor("idxp", [128, nt * 4 * idxcols], I16,
                          kind="ExternalInput").ap()
    rowp = nc.dram_tensor("rowp", [128, nt * cs4], BF16,
                          kind="ExternalInput").ap()
    valp = nc.dram_tensor("valp", [128, nt * cs4], BF16,
                          kind="ExternalInput").ap()
    cnts = nc.dram_tensor("cnts", [1, nt * 4], mybir.dt.int32,
                          kind="ExternalInput").ap()
    out = nc.dram_tensor("out", [nt * 128, out_dim], F32, kind="ExternalOutput").ap()
    secs = [nc.dram_tensor(f"support{s}", [sec_rows, out_dim], BF16).ap()
            for s in range(4)]

    with tile.TileContext(nc) as tc, tc.tile_pool(name="consts", bufs=1) as consts:
        w_sb = consts.tile([128, kb, out_dim], BF16)
        bp_sb = consts.tile([128, out_dim], BF16)
        shift_sb = consts.tile([128, out_dim], BF16)
        ones0_sb = consts.tile([128, 128], BF16)
        iota_sb = consts.tile([128, 128], BF16)
        gts = consts.tile([128, NGBUF, cs4, out_dim], BF16)
        for i in range(NGBUF):
            nc.vector.memset(gts[:, i], 0.0)
        for k in range(kb):
            nc.sync.dma_start(w_sb[:, k, :], Wp[k * 128:(k + 1) * 128, :])
        nc.sync.dma_start(bp_sb[:], bp[:])
        nc.sync.dma_start(shift_sb[:], shiftb[:])
        nc.sync.dma_start(ones0_sb[:], ones0_in[:])
        nc.sync.dma_start(iota_sb[:], iota_in[:])
        cnt_sb = consts.tile([128, nt * 4], mybir.dt.int32)
        nc.sync.dma_start(cnt_sb[0:1, :], cnts[:])

        # Phase-2 pools opened first: disjoint SBUF from phase-1 pools, so
        # phase-2 slab loads / sel builds carry no WAR deps on phase-1 frees.
        with (
            tc.tile_pool(name="slabs", bufs=2) as slabs,
            tc.tile_pool(name="sel", bufs=2) as selpool,
            tc.tile_pool(name="gelu", bufs=4) as gelupool,
            tc.tile_pool(name="p2psum", bufs=6, space="PSUM") as p2psum,
        ):
            with (
                tc.tile_pool(name="xt", bufs=2) as xpool,
                tc.tile_pool(name="p1psum", bufs=2, space="PSUM") as p1psum,
                tc.tile_pool(name="p1out", bufs=2) as p1out,
            ):
                for s4 in range(4):
                    for st in range(nsup_sec):
                        gcol = (s4 * nsup_sec + st) * xcols
                        sti = s4 * nsup_sec + st
                        xt = xpool.tile([128, kb, xcols], BF16)
                        for k in range(kb):
                            o = (sti * kb + k) * xcols
                            nc.sync.dma_start(xt[:, k, :], xTa[:, o:o + xcols])
                        so = p1out.tile([128, jt, out_dim], BF16)
                        for j in range(jt):
                            ps = p1psum.tile([128, out_dim], F32)
                            for k in range(kb):
                                nc.tensor.matmul(
                                    ps[:], lhsT=xt[:, k, j * 128:(j + 1) * 128],
                                    rhs=w_sb[:, k, :],
                                    start=(k == 0), stop=False)
                            nc.tensor.matmul(ps[:], lhsT=ones0_sb[:], rhs=bp_sb[:],
                                             start=False, stop=True)
                            nc.scalar.activation(so[:, j, :], ps[:], AF.Copy)
                        jh = jt // 2
                        r0 = st * xcols
                        full = secs[s4][r0:r0 + xcols, :] \
                            .rearrange("(p j) c -> p j c", p=128)
                        nc.scalar.dma_start(full[:, :jh, :], so[:, :jh, :])
                        nc.scalar.dma_start(full[:, jh:, :], so[:, jh:, :])

            nreg = nc.gpsimd.alloc_register("gcnt")

            def load_slab(sl):
                idx_sb = slabs.tile([128, tps * 4 * idxcols], I16, tag="idx")
                row_sb = slabs.tile([128, tps * cs4], BF16, tag="row")
                val_sb = slabs.tile([128, tps * cs4], BF16, tag="val")
                t0 = sl * tps
                nc.sync.dma_start(
                    idx_sb[:], idxp[:, t0 * 4 * idxcols:(t0 + tps) * 4 * idxcols])
                nc.sync.dma_start(
                    row_sb[:], rowp[:, t0 * cs4:(t0 + tps) * cs4])
                nc.sync.dma_start(
                    val_sb[:], valp[:, t0 * cs4:(t0 + tps) * cs4])
                return idx_sb, row_sb, val_sb

            nslab = nt // tps
            pending = load_slab(0)
            for sl in range(nslab):
                idx_sb, row_sb, val_sb = pending
                if sl + 1 < nslab:
                    pending = load_slab(sl + 1)
                t0 = sl * tps
                for tt in range(tps):
                    t = t0 + tt
                    # sel[p, c, d] = (row[p, c] == d) * val[p, c]
                    sel = selpool.tile([128, cs4, 128], BF16)
                    row3 = row_sb[:, tt * cs4:(tt + 1) * cs4].unsqueeze(2) \
                        .to_broadcast([128, cs4, 128])
                    val3 = val_sb[:, tt * cs4:(tt + 1) * cs4].unsqueeze(2) \
                        .to_broadcast([128, cs4, 128])
                    iota3 = iota_sb[:].unsqueeze(1).to_broadcast([128, cs4, 128])
                    nc.vector.tensor_tensor(sel[:], row3, iota3, op=ALU.is_equal)
                    nc.vector.tensor_tensor(sel[:], sel[:], val3, op=ALU.mult)
                    gt = gts[:, t % NGBUF]
                    for s in range(4):
                        g = t * 4 + s
                        nc.gpsimd.reg_load(nreg, cnt_sb[0:1, g:g + 1])
                        nc.gpsimd.dma_gather(
                            out_ap=gt[:, s * c_sub:(s + 1) * c_sub, :],
                            in_ap=secs[s][:],
                            idxs_ap=idx_sb[:, (tt * 4 + s) * idxcols:
                                           (tt * 4 + s + 1) * idxcols],
                            num_idxs=nidx,
                            num_idxs_reg=nreg,
                            elem_size=out_dim,
                            single_packet=False,
                            queue_num=(t * 4 + s) % 4,
                        )
                    ps = p2psum.tile([128, out_dim], F32)
                    for u in range(cs4):
                        nc.tensor.matmul(ps[:], lhsT=sel[:, u, :], rhs=gt[:, u, :],
                                         start=(u == 0), stop=False)
                    # folded BN shift: psum += e0.T @ broadcast(shift)
                    nc.tensor.matmul(ps[:], lhsT=ones0_sb[:], rhs=shift_sb[:],
                                     start=False, stop=True)
                    ob = gelupool.tile([128, out_dim], F32)
                    nc.scalar.activation(ob[:], ps[:], AF.Gelu)
                    nc.scalar.dma_start(out[t * 128:(t + 1) * 128, :], ob[:])

    nc.compile()
    return nc


def _preprocess(x, edge_row, edge_col, edge_val, W, b, gamma, beta,
                running_mean, running_var, bn_eps=1e-5):
    n, in_dim = x.shape
    out_dim = W.shape[1]
    npad = ((n + 4 * XCOLS - 1) // (4 * XCOLS)) * (4 * XCOLS)
    sec_rows = npad // 4
    assert sec_rows <= 32768, "support section must be int16-addressable"
    shard = n // N_CORES
    assert shard * N_CORES == n
    nt = (shard + 127) // 128
    nt = ((nt + TPS - 1) // TPS) * TPS

    inv_std = 1.0 / np.sqrt(running_var.astype(np.float64) + bn_eps)
    scale = (inv_std * gamma.astype(np.float64)).astype(np.float32)
    shift = (beta.astype(np.float64) - running_mean.astype(np.float64) * inv_std
             * gamma.astype(np.float64)).astype(np.float32)

    # xTa[p, st, k, c] = x[node(st, c), k*128 + p]  (contiguous per partition
    # per supertile); node(st, c) = st*xcols + c
    nsup = npad // XCOLS
    xpadT = np.zeros((in_dim, npad), np.float32)
    xpadT[:, :n] = x.T
    xTa = np.ascontiguousarray(
        xpadT.reshape(in_dim // 128, 128, nsup, XCOLS)
        .transpose(1, 2, 0, 3).reshape(128, nsup * in_dim // 128 * XCOLS)
    ).astype(BF)
    Wp = (W * scale[None, :]).astype(BF)
    bp = np.ascontiguousarray(
        np.broadcast_to((b * scale).astype(np.float32), (128, out_dim))).astype(BF)
    shiftb = np.ascontiguousarray(np.broadcast_to(shift, (128, out_dim))).astype(BF)
    ones0 = np.zeros((128, 128), BF)
    ones0[0, :] = 1
    iota = np.ascontiguousarray(
        np.broadcast_to(np.arange(128, dtype=np.float32), (128, 128))).astype(BF)

    per_core = []
    c_sub = 1
    for m in range(N_CORES):
        lo, hi = m * shard, (m + 1) * shard
        mask = (edge_row >= lo) & (edge_row < hi)
        er = (edge_row[mask] - lo).astype(np.int64)
        ec = edge_col[mask].astype(np.int64)
        ev = edge_val[mask].astype(np.float32)
        tile_of = er >> 7
        sec_of = ec // sec_rows
        gid = tile_of * 4 + sec_of          # tile-major, section-minor
        order = np.argsort(gid, kind="stable")
        er, ec, ev, gid = er[order], ec[order], ev[order], gid[order]
        counts = np.bincount(gid, minlength=4 * nt)
        per_core.append((er, ec, ev, gid, counts))
        c_sub = max(c_sub, int(((counts + 127) // 128).max()))
    nidx = c_sub * 128
    idxcols = nidx // 16
    ng = 4 * nt

    in_maps = []
    for m in range(N_CORES):
        er, ec, ev, gid, counts = per_core[m]
        starts = np.zeros(ng, np.int64)
        np.cumsum(counts[:-1], out=starts[1:])
        rank = np.arange(len(er)) - starts[gid]
        # group g = t*4+s occupies chunk columns [g*c_sub, (g+1)*c_sub)
        rowp = np.zeros((128, ng * c_sub), BF)
        valp = np.zeros((128, ng * c_sub), BF)
        rowp[rank & 127, gid * c_sub + (rank >> 7)] = (er & 127).astype(BF)
        valp[rank & 127, gid * c_sub + (rank >> 7)] = ev.astype(BF)
        # support row permutation: node c (within section) lives at row
        # r = st*XCOLS + (c % 128)*jt + (c % XCOLS) // 128  (write-contiguous)
        jt = XCOLS // 128
        csec = ec % sec_rows
        rsec = ((csec // XCOLS) * XCOLS + (csec % 128) * jt
                + (csec % XCOLS) // 128)
        idx16 = np.full((16, ng * idxcols), -1, np.int16)
        idx16[rank & 15, gid * idxcols + (rank >> 4)] = rsec.astype(np.int16)
        cnts_arr = counts.astype(np.int32)
        empty = np.nonzero(cnts_arr == 0)[0]
        if len(empty):
            idx16[0, empty * idxcols] = 0
            cnts_arr[empty] = 1
        in_maps.append({
            "xTa": xTa, "Wp": Wp, "bp": bp, "shiftb": shiftb, "ones0": ones0,
            "iota": iota, "cnts": cnts_arr.reshape(1, ng),
            "idxp": np.ascontiguousarray(np.tile(idx16, (8, 1))),
            "rowp": np.ascontiguousarray(rowp),
            "valp": np.ascontiguousarray(valp),
        })

    params = dict(in_dim=in_dim, out_dim=out_dim, npad=npad,
                  nt=nt, c_sub=c_sub, tps=TPS, xcols=XCOLS)
    return in_maps, params, shard


def kernel(x, edge_row, edge_col, edge_val, W, b, gamma, beta,
           running_mean, running_var):
    x = np.asarray(x)
    edge_row = np.asarray(edge_row)
    edge_col = np.asarray(edge_col)
    edge_val = np.asarray(edge_val)
    W = np.asarray(W)
    b = np.asarray(b)
    gamma = np.asarray(gamma)
    beta = np.asarray(beta)
    running_mean = np.asarray(running_mean)
    running_var = np.asarray(running_var)

    in_maps, params, shard = _preprocess(
        x, edge_row, edge_col, edge_val, W, b, gamma, beta,
        running_mean, running_var)
    nc = _build_program(**params)
    res = run_bass_kernel_spmd(nc, in_maps, core_ids=list(range(N_CORES)))
    outs = [res.results[m]["out"][:shard] for m in range(N_CORES)]
    return np.concatenate(outs, axis=0).astype(np.float32)


# revision 4
# speedup vs baseline: 1.3553x; 1.3553x over previous
"""GCN layer (Linear -> weighted-adjacency SpMM -> BatchNorm(eval) -> exact GELU)
as a Bass/Tile kernel on 8 Trainium2 NeuronCores.

Sharding: destination-node rows are sharded across the 8 cores (12500 rows
each); edges partitioned by destination row; W/b/BN params replicated.  Each
core computes the full `support = x @ W' + b'` redundantly in bf16 (phase 1:
cheaper than collectives at this size), writing it to HBM as 4
int16-addressable row-sections.  Phase 2 walks destination tiles (128 rows
each): per tile, four dma_gathers (one per section, per-gather edge count in a
runtime register trimming descriptor generation to real edges) fetch source
rows, and 4*c_sub one-hot selector matmuls plus one folded BN-shift matmul and
one bias matmul accumulate the entire segment-sum into a single PSUM group.

Engine allocation is strictly partitioned so no in-order queue ever blocks
another: PE does all matmuls; DVE only builds selectors; the scalar engine
drains PSUM (phase-1 copy, phase-2 exact GELU) and issues support/output
writes on its own HWDGE ring (parallel to the sync ring doing loads); gpsimd
preps gathers round-robin across all 4 SWDGE queues (queue = source section,
so a queue only starts once phase 1 has produced its section - measured 4x
descriptor throughput vs a single queue, which is the kernel's floor:
~400k edges/core x ~3.3ns/descriptor).

Host-side prep inside kernel(): repack x into a per-supertile
contiguous-per-partition layout; permute support rows so phase-1 writes are
contiguous per partition (the gather indices absorb the permutation); group
edges by (dest-tile, source-section) and pack per-group edge row/val into
fixed 128-edge chunk layout plus 16-partition-wrapped int16 gather indices.
BN is folded on the host (W' = W * s, shift = beta - mean * s,
s = gamma / sqrt(var + eps)).  One SPMD program serves all 8 cores.
"""

import sys

sys.path.insert(0, "/opt/trn_rl_repo")

import numpy as np
import ml_dtypes

import concourse.tile as tile
from concourse import bacc, mybir
from concourse.bass_utils import run_bass_kernel_spmd

F32 = mybir.dt.float32
BF16 = mybir.dt.bfloat16
I16 = mybir.dt.int16
AF = mybir.ActivationFunctionType
ALU = mybir.AluOpType

BF = ml_dtypes.bfloat16

N_CORES = 8
TPS = 14      # dest tiles per index-slab load (must divide nt)
XCOLS = 3584  # node columns per phase-1 supertile (divides sec_rows)
NGBUF = 4     # round-robin gather buffers (one per dest tile in flight)


def _build_program(*, in_dim, out_dim, npad, nt, c_sub, tps, xcols):
    assert in_dim % 128 == 0 and npad % (4 * xcols) == 0
    assert nt % tps == 0
    kb = in_dim // 128
    sec_rows = npad // 4
    nsup_sec = sec_rows // xcols
    jt = xcols // 128
    nidx = c_sub * 128
    idxcols = nidx // 16
    cs4 = 4 * c_sub

    nc = bacc.Bacc("TRN2", target_bir_lowering=False, debug=False,
                   num_devices=N_CORES, num_swdge_queues=4)

    nsup = npad // xcols
    xTa = nc.dram_tensor("xTa", [128, nsup * in_dim // 128 * xcols], BF16,
                         kind="ExternalInput").ap()
    Wp = nc.dram_tensor("Wp", [in_dim, out_dim], BF16, kind="ExternalInput").ap()
    bp = nc.dram_tensor("bp", [128, out_dim], BF16, kind="ExternalInput").ap()
    shiftb = nc.dram_tensor("shiftb", [128, out_dim], BF16, kind="ExternalInput").ap()
    ones0_in = nc.dram_tensor("ones0", [128, 128], BF16, kind="ExternalInput").ap()
    iota_in = nc.dram_tensor("iota", [128, 128], BF16, kind="ExternalInput").ap()
    idxp = nc.dram_tensor("idxp", [128, nt * 4 * idxcols], I16,
                          kind="ExternalInput").ap()
    rowp = nc.dram_tensor("rowp", [128, nt * cs4], BF16,
                          kind="ExternalInput").ap()
    valp = nc.dram_tensor("valp", [128, nt * cs4], BF16,
                          kind="ExternalInput").ap()
    cnts = nc.dram_tensor("cnts", [1, nt * 4], mybir.dt.int32,
                          kind="ExternalInput").ap()
    out = nc.dram_tensor("out", [nt * 128, out_dim], F32, kind="ExternalOutput").ap()
    secs = [nc.dram_tensor(f"support{s}", [sec_rows, out_dim], BF16).ap()
            for s in range(4)]

    with tile.TileContext(nc) as tc, tc.tile_pool(name="consts", bufs=1) as consts:
        w_sb = consts.tile([128, kb, out_dim], BF16)
        bp_sb = consts.tile([128, out_dim], BF16)
        shift_sb = consts.tile([128, out_dim], BF16)
        ones0_sb = consts.tile([128, 128], BF16)
        iota_sb = consts.tile([128, 128], BF16)
        gts = consts.tile([128, NGBUF, cs4, out_dim], BF16)
        for i in range(NGBUF):
            nc.vector.memset(gts[:, i], 0.0)
        for k in range(kb):
            nc.sync.dma_start(w_sb[:, k, :], Wp[k * 128:(k + 1) * 128, :])
        nc.sync.dma_start(bp_sb[:], bp[:])
        nc.sync.dma_start(shift_sb[:], shiftb[:])
        nc.sync.dma_start(ones0_sb[:], ones0_in[:])
        nc.sync.dma_start(iota_sb[:], iota_in[:])
        cnt_sb = consts.tile([128, nt * 4], mybir.dt.int32)
        nc.sync.dma_start(cnt_sb[0:1, :], cnts[:])

        # Phase-2 pools opened first: disjoint SBUF from phase-1 pools, so
        # phase-2 slab loads / sel builds carry no WAR deps on phase-1 frees.
        with (
            tc.tile_pool(name="slabs", bufs=2) as slabs,
            tc.tile_pool(name="sel", bufs=2) as selpool,
            tc.tile_pool(name="gelu", bufs=4) as gelupool,
            tc.tile_pool(name="p2psum", bufs=6, space="PSUM") as p2psum,
        ):
            with (
                tc.tile_pool(name="xt", bufs=2) as xpool,
                tc.tile_pool(name="p1psum", bufs=2, space="PSUM") as p1psum,
                tc.tile_pool(name="p1out", bufs=2) as p1out,
            ):
                for s4 in range(4):
                    for st in range(nsup_sec):
                        gcol = (s4 * nsup_sec + st) * xcols
                        sti = s4 * nsup_sec + st
                        xt = xpool.tile([128, kb, xcols], BF16)
                        for k in range(kb):
                            o = (sti * kb + k) * xcols
                            nc.sync.dma_start(xt[:, k, :], xTa[:, o:o + xcols])
                        so = p1out.tile([128, jt, out_dim], BF16)
                        for j in range(jt):
                            ps = p1psum.tile([128, out_dim], F32)
                            for k in range(kb):
                                nc.tensor.matmul(
                                    ps[:], lhsT=xt[:, k, j * 128:(j + 1) * 128],
                                    rhs=w_sb[:, k, :],
                                    start=(k == 0), stop=False)
                            nc.tensor.matmul(ps[:], lhsT=ones0_sb[:], rhs=bp_sb[:],
                                             start=False, stop=True)
                            nc.scalar.activation(so[:, j, :], ps[:], AF.Copy)
                        jh = jt // 2
                        r0 = st * xcols
                        full = secs[s4][r0:r0 + xcols, :] \
                            .rearrange("(p j) c -> p j c", p=128)
                        nc.scalar.dma_start(full[:, :jh, :], so[:, :jh, :])
                        nc.scalar.dma_start(full[:, jh:, :], so[:, jh:, :])

            nreg = nc.gpsimd.alloc_register("gcnt")

            def load_slab(sl):
                idx_sb = slabs.tile([128, tps * 4 * idxcols], I16, tag="idx")
                row_sb = slabs.tile([128, tps * cs4], BF16, tag="row")
                val_sb = slabs.tile([128, tps * cs4], BF16, tag="val")
                t0 = sl * tps
                nc.sync.dma_start(
                    idx_sb[:], idxp[:, t0 * 4 * idxcols:(t0 + tps) * 4 * idxcols])
                nc.sync.dma_start(
                    row_sb[:], rowp[:, t0 * cs4:(t0 + tps) * cs4])
                nc.sync.dma_start(
                    val_sb[:], valp[:, t0 * cs4:(t0 + tps) * cs4])
                return idx_sb, row_sb, val_sb

            nslab = nt // tps
            pending = load_slab(0)
            for sl in range(nslab):
                idx_sb, row_sb, val_sb = pending
                if sl + 1 < nslab:
                    pending = load_slab(sl + 1)
                t0 = sl * tps
                for tt in range(tps):
                    t = t0 + tt
                    # sel[p, c, d] = (row[p, c] == d) * val[p, c]
                    sel = selpool.tile([128, cs4, 128], BF16)
                    row3 = row_sb[:, tt * cs4:(tt + 1) * cs4].unsqueeze(2) \
                        .to_broadcast([128, cs4, 128])
                    val3 = val_sb[:, tt * cs4:(tt + 1) * cs4].unsqueeze(2) \
                        .to_broadcast([128, cs4, 128])
                    iota3 = iota_sb[:].unsqueeze(1).to_broadcast([128, cs4, 128])
                    nc.vector.tensor_tensor(sel[:], row3, iota3, op=ALU.is_equal)
                    nc.vector.tensor_tensor(sel[:], sel[:], val3, op=ALU.mult)
                    gt = gts[:, t % NGBUF]
                    for s in range(4):
                        g = t * 4 + s
                        nc.gpsimd.reg_load(nreg, cnt_sb[0:1, g:g + 1])
                        nc.gpsimd.dma_gather(
                            out_ap=gt[:, s * c_sub:(s + 1) * c_sub, :],
                            in_ap=secs[s][:],
                            idxs_ap=idx_sb[:, (tt * 4 + s) * idxcols:
                                           (tt * 4 + s + 1) * idxcols],
                            num_idxs=nidx,
                            num_idxs_reg=nreg,
                            elem_size=out_dim,
                            single_packet=False,
                            queue_num=(t * 4 + s) % 4,
                        )
                    ps = p2psum.tile([128, out_dim], F32)
                    for u in range(cs4):
                        nc.tensor.matmul(ps[:], lhsT=sel[:, u, :], rhs=gt[:, u, :],
                                         start=(u == 0), stop=False)
                    # folded BN shift: psum += e0.T @ broadcast(shift)
                    nc.tensor.matmul(ps[:], lhsT=ones0_sb[:], rhs=shift_sb[:],
                                     start=False, stop=True)
                    ob = gelupool.tile([128, out_dim], F32)
                    nc.scalar.activation(ob[:], ps[:], AF.Gelu)
                    nc.scalar.dma_start(out[t * 128:(t + 1) * 128, :], ob[:])

    nc.compile()
    return nc


def _preprocess(x, edge_row, edge_col, edge_val, W, b, gamma, beta,
                running_mean, running_var, bn_eps=1e-5):
    n, in_dim = x.shape
    out_dim = W.shape[1]
    npad = ((n + 4 * XCOLS - 1) // (4 * XCOLS)) * (4 * XCOLS)
    sec_rows = npad // 4
    assert sec_rows <= 32768, "support section must be int16-addressable"
    shard = n // N_CORES
    assert shard * N_CORES == n
    nt = (shard + 127) // 128
    nt = ((nt + TPS - 1) // TPS) * TPS

    inv_std = 1.0 / np.sqrt(running_var.astype(np.float64) + bn_eps)
    scale = (inv_std * gamma.astype(np.float64)).astype(np.float32)
    shift = (beta.astype(np.float64) - running_mean.astype(np.float64) * inv_std
             * gamma.astype(np.float64)).astype(np.float32)

    # xTa[p, st, k, c] = x[node(st, c), k*128 + p]  (contiguous per partition
    # per supertile); node(st, c) = st*xcols + c
    nsup = npad // XCOLS
    xpadT = np.zeros((in_dim, npad), np.float32)
    xpadT[:, :n] = x.T
    xTa = np.ascontiguousarray(
        xpadT.reshape(in_dim // 128, 128, nsup, XCOLS)
        .transpose(1, 2, 0, 3).reshape(128, nsup * in_dim // 128 * XCOLS)
    ).astype(BF)
    Wp = (W * scale[None, :]).astype(BF)
    bp = np.ascontiguousarray(
        np.broadcast_to((b * scale).astype(np.float32), (128, out_dim))).astype(BF)
    shiftb = np.ascontiguousarray(np.broadcast_to(shift, (128, out_dim))).astype(BF)
    ones0 = np.zeros((128, 128), BF)
    ones0[0, :] = 1
    iota = np.ascontiguousarray(
        np.broadcast_to(np.arange(128, dtype=np.float32), (128, 128))).astype(BF)

    per_core = []
    c_sub = 1
    for m in range(N_CORES):
        lo, hi = m * shard, (m + 1) * shard
        mask = (edge_row >= lo) & (edge_row < hi)
        er = (edge_row[mask] - lo).astype(np.int64)
        ec = edge_col[mask].astype(np.int64)
        ev = edge_val[mask].astype(np.float32)
        tile_of = er >> 7
        sec_of = ec // sec_rows
        gid = tile_of * 4 + sec_of          # tile-major, section-minor
        order = np.argsort(gid, kind="stable")
        er, ec, ev, gid = er[order], ec[order], ev[order], gid[order]
        counts = np.bincount(gid, minlength=4 * nt)
        per_core.append((er, ec, ev, gid, counts))
        c_sub = max(c_sub, int(((counts + 127) // 128).max()))
    nidx = c_sub * 128
    idxcols = nidx // 16
    ng = 4 * nt

    in_maps = []
    for m in range(N_CORES):
        er, ec, ev, gid, counts = per_core[m]
        starts = np.zeros(ng, np.int64)
        np.cumsum(counts[:-1], out=starts[1:])
        rank = np.arange(len(er)) - starts[gid]
        # group g = t*4+s occupies chunk columns [g*c_sub, (g+1)*c_sub)
        rowp = np.zeros((128, ng * c_sub), BF)
        valp = np.zeros((128, ng * c_sub), BF)
        rowp[rank & 127, gid * c_sub + (rank >> 7)] = (er & 127).astype(BF)
        valp[rank & 127, gid * c_sub + (rank >> 7)] = ev.astype(BF)
        # support row permutation: node c (within section) lives at row
        # r = st*XCOLS + (c % 128)*jt + (c % XCOLS) // 128  (write-contiguous)
        jt = XCOLS // 128
        csec = ec % sec_rows
        rsec = ((csec // XCOLS) * XCOLS + (csec % 128) * jt
                + (csec % XCOLS) // 128)
        idx16 = np.full((16, ng * idxcols), -1, np.int16)
        idx16[rank & 15, gid * idxcols + (rank >> 4)] = rsec.astype(np.int16)
        cnts_arr = counts.astype(np.int32)
        empty = np.nonzero(cnts_arr == 0)[0]
        if len(empty):
            idx16[0, empty * idxcols] = 0
            cnts_arr[empty] = 1
        in_maps.append({
            "xTa": xTa, "Wp": Wp, "bp": bp, "shiftb": shiftb, "ones0": ones0,
            "iota": iota, "cnts": cnts_arr.reshape(1, ng),
            "idxp": np.ascontiguousarray(np.tile(idx16, (8, 1))),
            "rowp": np.ascontiguousarray(rowp),
            "valp": np.ascontiguousarray(valp),
        })

    params = dict(in_dim=in_dim, out_dim=out_dim, npad=npad,
                  nt=nt, c_sub=c_sub, tps=TPS, xcols=XCOLS)
    return in_maps, params, shard


def kernel(x, edge_row, edge_col, edge_val, W, b, gamma, beta,
           running_mean, running_var):
    x = np.asarray(x)
    edge_row = np.asarray(edge_row)
    edge_col = np.asarray(edge_col)
    edge_val = np.asarray(edge_val)
    W = np.asarray(W)
    b = np.asarray(b)
    gamma = np.asarray(gamma)
    beta = np.asarray(beta)
    running_mean = np.asarray(running_mean)
    running_var = np.asarray(running_var)

    in_maps, params, shard = _preprocess(
        x, edge_row, edge_col, edge_val, W, b, gamma, beta,
        running_mean, running_var)
    nc = _build_program(**params)
    res = run_bass_kernel_spmd(nc, in_maps, core_ids=list(range(N_CORES)))
    outs = [res.results[m]["out"][:shard] for m in range(N_CORES)]
    return np.concatenate(outs, axis=0).astype(np.float32)
